# revision 65
# baseline (speedup 1.0000x reference)
"""Trainium2 Bass kernel for causal self-attention (GQA, RoPE, q/k-RMSNorm).

Sharding: tensor-parallel over heads across 8 cores.
  - core c owns q-heads [4c, 4c+4) and kv-head c//2
  - x^T is pre-transposed on the host and DMA'd in bf16, so there is no
    on-device transpose phase; V is projected directly into its natural
    [S, D] layout by using x^T as the matmul lhsT
  - all projections/attention matmuls run in bf16 (fp32 PSUM); the causal
    mask is injected as an exact fp8 DoubleRow PE bias matmul (0 / -240
    tiles) accumulated into the score PSUM before the exp
  - QKV streams through a 2-bank PSUM scratch so the attention pools can
    stay resident; rmsnorm+rope is restructured so the reduce chain
    (sq->ssq->sqrt->recip->PE broadcast) and the rope chain (on raw
    values) run independently and meet in one final multiply
  - o_proj is a row-sharded partial (woT blocks per local head); each
    t-chunk's [2048, 512] partial is summed across cores by its own bf16
    ReduceScatter, whose rank shard is exactly this core's outT columns,
    so chunk j's collective overlaps chunk j+1's compute
"""

import sys

sys.path.insert(0, "/opt/trn_rl_repo")

from contextlib import ExitStack

import numpy as np

import bass_rust
import concourse.bass as bass
import concourse.mybir as mybir
from concourse import tile

F32 = mybir.dt.float32
F32R = mybir.dt.float32r
BF16 = mybir.dt.bfloat16
FP8 = mybir.dt.float8e4
FP8E5 = mybir.dt.float8e5
DR = mybir.MatmulPerfMode.DoubleRow

N_HEAD = 32
N_KV = 4
D = 128
C = 2048
T = 2048
NCORES = 8
HPC = N_HEAD // NCORES  # q heads per core = 4
THETA = 1000000.0
EPS = 1e-6
SCALE = 1.0 / np.sqrt(128.0)
MASKVAL = -240.0  # pre-scale bias; * SCALE = -21 -> exp ~ 6e-10 -> fp8 0

NT = T // 512  # 4 T-chunks of 512
NK = 16  # contraction tiles of 128
NS = T // 128  # 16 s-blocks of 128

# stream_shuffle swaps within each 32-partition quadrant; adjacent-pair swap
SWAP_MASK = [i ^ 1 for i in range(32)]

_NPDT = None


def _npdt():
    global _NPDT
    if _NPDT is None:
        import ml_dtypes

        _NPDT = (
            np.dtype(ml_dtypes.bfloat16),
            np.dtype(ml_dtypes.float8_e4m3),
            np.dtype(ml_dtypes.float8_e5m2),
        )
    return _NPDT


def split_multiwaits(nc):
    """The walrus build in this container supports one sync-wait per
    instruction; hoist extra waits onto NOPs inserted before the offender."""
    ctr = 0
    for f in nc.m.functions:
        for bb in f.blocks:
            new_insts = []
            changed = False
            for inst in bb.instructions:
                si = inst.sync_info
                if si is not None and si.on_wait and len(si.on_wait) > 1:
                    waits = list(si.on_wait)
                    for w in waits[:-1]:
                        ctr += 1
                        nop = bass_rust.InstNoOp(name=f"splitw-{ctr}", ins=[], outs=[])
                        nop.engine = inst.engine
                        nop.sync_info = bass_rust.SyncInfo(on_wait=[w], on_update=[])
                        new_insts.append(nop)
                    inst.sync_info = bass_rust.SyncInfo(
                        on_wait=[waits[-1]], on_update=list(si.on_update or [])
                    )
                    changed = True
                new_insts.append(inst)
            if changed:
                bb.instructions = new_insts


def _bcast(ap, n):
    """Insert a stride-0 free dim of extent n after the partition dim."""
    import dataclasses
    assert len(ap.ap) == 2
    return dataclasses.replace(ap, ap=[ap.ap[0], [0, n], ap.ap[1]])


def build_program(bench_reps=0, phases="ABDF", split=True):
    nc = bass.Bass("TRN2", target_bir_lowering=False, debug=False, num_devices=NCORES)

    xtb = nc.declare_dram_parameter("xtb", [128, NK * T], BF16, isOutput=False)
    wqb = nc.declare_dram_parameter(
        "wqb", [128, 5 * NK * 128], BF16, isOutput=False
    )
    wvb = nc.declare_dram_parameter("wvb", [128, NK * 128], BF16, isOutput=False)
    wob = nc.declare_dram_parameter(
        "wob", [128, HPC * 16 * 128], BF16, isOutput=False
    )
    cost = nc.declare_dram_parameter("cost", [128, T], BF16, isOutput=False)
    sint = nc.declare_dram_parameter("sint", [128, T], BF16, isOutput=False)
    ident8p = nc.declare_dram_parameter("ident8p", [128, 2 * 128], FP8, isOutput=False)
    mask8p = nc.declare_dram_parameter(
        "mask8p", [128, 2 * 2 * 2 * 512], FP8, isOutput=False
    )
    outT = nc.declare_dram_parameter("outT", [256, T], BF16, isOutput=True)

    rg = [list(range(NCORES))]
    collectives = bench_reps == 0

    with tile.TileContext(nc) as tc, ExitStack() as ctx:
        const = ctx.enter_context(tc.tile_pool(name="const", bufs=1))
        wpool = ctx.enter_context(tc.tile_pool(name="wpool", bufs=1))
        act = ctx.enter_context(tc.tile_pool(name="act", bufs=1))
        dram = ctx.enter_context(tc.tile_pool(name="dram", bufs=1, space="DRAM"))

        # ---- constants ----
        ones_col = const.tile([128, 1], BF16)
        nc.vector.memset(ones_col[:], 1.0)
        ones_row = const.tile([1, 128], BF16)
        nc.vector.memset(ones_row[:], 1.0)
        eps_col = const.tile([128, 1], F32)
        nc.vector.memset(eps_col[:], EPS)
        ident8 = const.tile([128, 2, 128], FP8)
        nc.sync.dma_start(ident8[:], ident8p[:, :])
        mask8 = const.tile([128, 2, 2, 2, 512], FP8)
        nc.sync.dma_start(mask8[:], mask8p[:, :])

        # ---- resident weights / tables ----
        wq_sb = wpool.tile([128, 5, NK, 128], BF16)  # q0..q3 + k
        for o in range(5):
            nc.sync.dma_start(
                wq_sb[:, o, :, :], wqb[:, o * NK * 128:(o + 1) * NK * 128]
            )
        xt_sb = wpool.tile([128, NK, T], BF16)
        for kt in range(NK):
            nc.sync.dma_start(
                xt_sb[:, kt, :], xtb[:, kt * T:(kt + 1) * T]
            )
        wv_sb = wpool.tile([128, NK, 128], BF16)
        nc.sync.dma_start(wv_sb[:], wvb[:, :])
        wo_sb = wpool.tile([128, HPC, 16, 128], BF16)
        nc.sync.dma_start(wo_sb[:], wob[:, :])
        cos_sb = wpool.tile([128, T], BF16)
        nc.sync.dma_start(cos_sb[:], cost[:, :])
        sin_sb = wpool.tile([128, T], BF16)
        nc.sync.dma_start(sin_sb[:], sint[:, :])

        # ---- persistent activations ----
        kT = act.tile([128, T], BF16)
        vN = act.tile([128, NS, 128], BF16)  # natural [S,D] s-blocks
        yT = act.tile([128, HPC, T], BF16)

        # per-t-chunk partial-o + ReduceScatter buffers: a full-r RS of
        # [2048, 512] hands core c rows [256c, 256c+256) = its outT slice
        o_part = [
            dram.tile([16 * 128, 512], BF16, name=f"opart{j}") for j in range(NT)
        ]
        rs_out = [
            dram.tile([256, 512], BF16, name=f"rsout{j}") for j in range(NT)
        ]

        def qkv_chunk(j, pb_sb, ps_scr):
            js = slice(j * 512, (j + 1) * 512)
            raw = pb_sb.tile([128, 5, 512], BF16, tag="raw", bufs=2)
            for o in range(5):  # q0..q3, k -> [d, t] via 2-bank streaming
                acc = ps_scr.tile([128, 512], F32, tag="s", name=f"acc{o}")
                for kt in range(NK):
                    nc.tensor.matmul(
                        acc[:], wq_sb[:, o, kt, :], xt_sb[:, kt, js],
                        start=(kt == 0), stop=(kt == NK - 1),
                    )
                nc.scalar.activation(
                    raw[:, o, :], acc[:], mybir.ActivationFunctionType.Copy
                )
            # v directly in natural [s, d] layout: lhsT = x^T, rhs = Wv^T
            vacc = ps_scr.tile([128, 4, 128], F32, tag="v", bufs=1)
            for u in range(4):
                for kt in range(NK):
                    nc.tensor.matmul(
                        vacc[:, u, :],
                        xt_sb[:, kt, j * 512 + u * 128:j * 512 + (u + 1) * 128],
                        wv_sb[:, kt, :],
                        start=(kt == 0), stop=(kt == NK - 1),
                    )
            nc.vector.tensor_copy(vN[:, 4 * j:4 * j + 4, :], vacc[:])
            return raw

        def norms_reduce(j, raw, pc_sb, ps_scr):
            # sq -> ssq -> sqrt -> recip; emitted before the deferred o_proj
            # so the cross-engine latency hides under its PE work
            n = 5
            sqf = pc_sb.tile([128, n, 512], BF16, tag="sq", bufs=1)
            for i in range(n):
                nc.vector.tensor_mul(sqf[:, i, :], raw[:, i, :], raw[:, i, :])
            ssq = [ps_scr.tile([128, 512], F32, tag="s", name=f"ssq{i}")
                   for i in range(n)]
            for i in range(n):
                nc.tensor.matmul(ssq[i][0:1, :], ones_col[:], sqf[:, i, :])
            rms = pc_sb.tile([1, n, 512], BF16, tag="rms", bufs=2)
            for i in range(n):
                nc.scalar.activation(
                    rms[:, i, :], ssq[i][0:1, :],
                    mybir.ActivationFunctionType.Sqrt,
                    scale=1.0 / 128.0, bias=eps_col[0:1, :],
                )
            rinv = pc_sb.tile([1, n, 512], BF16, tag="ri", bufs=1)
            with nc.allow_low_precision(reason="feeds PE broadcast"):
                nc.vector.reciprocal(rinv[:], rms[:])
            return rinv

        def norms_finish(j, raw, rinv, pc_sb, ps_scr):
            # rope on raw values + one final multiply from the rb broadcast
            js = slice(j * 512, (j + 1) * 512)
            n = 5
            rb = [ps_scr.tile([128, 512], F32, tag="s", name=f"rb{i}")
                  for i in range(n)]
            for i in range(n):
                nc.tensor.matmul(rb[i][:], ones_row[:], rinv[:, i, :])
            qs = pc_sb.tile([128, n, 512], BF16, tag="qs", bufs=1)
            for i in range(n):
                nc.vector.stream_shuffle(qs[:, i, :], raw[:, i, :], mask=SWAP_MASK)
            t1 = pc_sb.tile([128, n, 512], BF16, tag="t1", bufs=1)
            for i in range(n):
                nc.gpsimd.tensor_mul(t1[:, i, :], raw[:, i, :], cos_sb[:, js])
            t2 = pc_sb.tile([128, n, 512], BF16, tag="t2", bufs=1)
            for i in range(n):
                nc.vector.tensor_mul(t2[:, i, :], qs[:, i, :], sin_sb[:, js])
            rr = pc_sb.tile([128, n, 512], BF16, tag="rr", bufs=1)
            nc.vector.tensor_add(rr[:], t1[:], t2[:])
            qTc = pc_sb.tile([128, HPC, 512], BF16, tag="qtc", bufs=2)
            for i in range(HPC):
                nc.vector.tensor_mul(qTc[:, i, :], rr[:, i, :], rb[i][:])
            nc.vector.tensor_mul(kT[:, js], rr[:, 4, :], rb[4][:])
            return qTc

        def attn_chunk(j, qTc, pd_sb, ps_att, ps_scr):
            js = slice(j * 512, (j + 1) * 512)
            npair = 2 * j + 2
            for h in range(HPC):
                ps_y = ps_att.tile([128, 512], F32, tag="psy")
                ps_den = ps_scr.tile([128, 512], F32, tag="s", name=f"den{h}")
                for p in range(npair):
                    ps_s = ps_att.tile([128, 2, 512], F32, tag="pss", bufs=2)
                    diag = p >= 2 * j
                    for w in range(2):
                        i = 2 * p + w
                        if diag:
                            nc.tensor.matmul(
                                ps_s[:, w, :], ident8[:],
                                mask8[:, p - 2 * j, w, :, :],
                                start=True, stop=False, perf_mode=DR,
                            )
                        nc.tensor.matmul(
                            ps_s[:, w, :],
                            kT[:, i * 128:(i + 1) * 128], qTc[:, h, :],
                            start=not diag, stop=True,
                        )
                    etb = pd_sb.tile([128, 2, 512], BF16, tag="etb", bufs=2)
                    nc.scalar.activation(
                        etb[:], ps_s[:], mybir.ActivationFunctionType.Exp,
                        scale=float(SCALE),
                    )
                    for w in range(2):
                        st = dict(
                            start=(p == 0 and w == 0),
                            stop=(p == npair - 1 and w == 1),
                        )
                        nc.tensor.matmul(
                            ps_y[:], vN[:, 2 * p + w, :], etb[:, w, :], **st
                        )
                        nc.tensor.matmul(
                            ps_den[0:1, :], ones_col[:], etb[:, w, :], **st
                        )
                rd = pd_sb.tile([1, 512], BF16, tag="rd", bufs=1)
                with nc.allow_low_precision(reason="feeds PE broadcast"):
                    nc.vector.reciprocal(rd[:], ps_den[0:1, :])
                ps_rb = ps_scr.tile([128, 512], F32, tag="s", name=f"psrb{h}")
                nc.tensor.matmul(ps_rb[:], ones_row[:], rd[:])
                rbc = pd_sb.tile([128, 512], BF16, tag="rbcd", bufs=2)
                nc.vector.tensor_copy(rbc[:], ps_rb[:])
                nc.vector.tensor_mul(yT[:, h, js], ps_y[:], rbc[:])

        def oproj_chunk(j, pf_sb, ps_scr):
            js = slice(j * 512, (j + 1) * 512)
            for rt in range(16):
                ps = ps_scr.tile([128, 512], F32, tag="s", name=f"pso{rt}")
                for h in range(HPC):
                    nc.tensor.matmul(
                        ps[:], wo_sb[:, h, rt, :], yT[:, h, js],
                        start=(h == 0), stop=(h == HPC - 1),
                    )
                ob = pf_sb.tile([128, 512], BF16, tag="ob", bufs=2)
                if rt % 2 == 0:
                    nc.vector.tensor_copy(ob[:], ps[:])
                else:
                    nc.scalar.activation(
                        ob[:], ps[:], mybir.ActivationFunctionType.Copy
                    )
                nc.sync.dma_start(o_part[j][rt * 128:(rt + 1) * 128, :], ob[:])
            if collectives:
                nc.gpsimd.collective_compute(
                    "ReduceScatter", mybir.AluOpType.add, replica_groups=rg,
                    ins=[o_part[j][:].opt()], outs=[rs_out[j][:].opt()],
                )

        def body():
            with tc.tile_pool(name="pb_sb", bufs=2) as pb_sb, \
                 tc.tile_pool(name="pc_sb", bufs=2) as pc_sb, \
                 tc.tile_pool(name="pd_sb", bufs=3) as pd_sb, \
                 tc.tile_pool(name="ps_att", bufs=1, space="PSUM") as ps_att, \
                 tc.tile_pool(name="ps_scr", bufs=2, space="PSUM") as ps_scr:
                for j in range(NT):
                    raw = qkv_chunk(j, pb_sb, ps_scr)
                    rinv = norms_reduce(j, raw, pc_sb, ps_scr)
                    if "D" in phases and "F" in phases and j > 0:
                        oproj_chunk(j - 1, pd_sb, ps_scr)
                    qTc = norms_finish(j, raw, rinv, pc_sb, ps_scr)
                    if "D" not in phases:
                        continue
                    attn_chunk(j, qTc, pd_sb, ps_att, ps_scr)
                if "D" in phases and "F" in phases:
                    oproj_chunk(NT - 1, pd_sb, ps_scr)

            # bounce the RS shards into outT
            if "F" not in phases or "D" not in phases:
                return
            with tc.tile_pool(name="po_sb", bufs=2) as po_sb:
                for j in range(NT):
                    js = slice(j * 512, (j + 1) * 512)
                    rsb = po_sb.tile([128, 2, 512], BF16, tag="rsb")
                    for m in range(2):
                        nc.sync.dma_start(
                            rsb[:, m, :], rs_out[j][m * 128:(m + 1) * 128, :]
                        )
                    for m in range(2):
                        nc.sync.dma_start(
                            outT[m * 128:(m + 1) * 128, js], rsb[:, m, :]
                        )

        if bench_reps:
            with tc.For_i(0, bench_reps, 1):
                body()
        else:
            body()

    if split:
        split_multiwaits(nc)
    return nc


# ---------------------------------------------------------------------------
# host side
# ---------------------------------------------------------------------------

_RUNNER_CACHE = None


def _make_runner(nc, n_cores=NCORES):
    """Build the sharded jit once; returns run(in_maps) -> list of out dicts."""
    import jax
    from jax.sharding import Mesh, NamedSharding, PartitionSpec
    from jax.experimental.shard_map import shard_map
    from concourse import bass2jax
    from concourse.bass2jax import _bass_exec_p, partition_id_tensor

    bass2jax.install_neuronx_cc_hook()

    partition_name = nc.partition_id_tensor.name if nc.partition_id_tensor else None
    in_names, out_names, out_avals, zero_outs = [], [], [], []
    for alloc in nc.m.functions[0].allocations:
        if not isinstance(alloc, mybir.MemoryLocationSet):
            continue
        name = alloc.memorylocations[0].name
        if alloc.kind == "ExternalInput":
            if name != partition_name:
                in_names.append(name)
        elif alloc.kind == "ExternalOutput":
            out_names.append(name)
            shape = tuple(alloc.tensor_shape)
            dtype = mybir.dt.np(alloc.dtype)
            out_avals.append(jax.core.ShapedArray(shape, dtype))
            zero_outs.append(np.zeros(shape, dtype))
    n_params = len(in_names)
    n_outs = len(out_avals)
    all_in_names = list(in_names) + list(out_names)
    if partition_name is not None:
        all_in_names.append(partition_name)
    donate = tuple(range(n_params, n_params + n_outs))

    def _body(*args):
        operands = list(args)
        if partition_name is not None:
            operands.append(partition_id_tensor())
        outs = _bass_exec_p.bind(
            *operands,
            out_avals=tuple(out_avals),
            in_names=tuple(all_in_names),
            out_names=tuple(out_names),
            lowering_input_output_aliases=(),
            sim_require_finite=True,
            sim_require_nnan=True,
            nc=nc,
        )
        return tuple(outs)

    devices = jax.devices()[:n_cores]
    mesh = Mesh(np.asarray(devices), ("core",))
    sharded = jax.jit(
        shard_map(
            _body, mesh=mesh,
            in_specs=(PartitionSpec("core"),) * (n_params + n_outs),
            out_specs=(PartitionSpec("core"),) * n_outs,
            check_rep=False,
        ),
        donate_argnums=donate,
        keep_unused=True,
    )
    shard = NamedSharding(mesh, PartitionSpec("core"))
    zshapes = [((n_cores * z.shape[0],) + z.shape[1:], z.dtype) for z in zero_outs]

    def run(in_maps):
        concat_in = [
            jax.device_put(
                np.concatenate(
                    [np.asarray(in_maps[c][n]) for c in range(n_cores)], axis=0
                ),
                shard,
            )
            for n in in_names
        ]
        zs = [jax.device_put(np.zeros(s, d), shard) for s, d in zshapes]
        outs = sharded(*concat_in, *zs)
        return [
            {
                name: np.asarray(outs[i]).reshape(n_cores, *out_avals[i].shape)[c]
                for i, name in enumerate(out_names)
            }
            for c in range(n_cores)
        ]

    return run


def _get_runner():
    global _RUNNER_CACHE
    if _RUNNER_CACHE is None:
        _RUNNER_CACHE = _make_runner(build_program())
    return _RUNNER_CACHE


def make_inputs(x, input_pos, Wq, Wk, Wv, Wo, q_norm_w, k_norm_w):
    """Host-side sharding / layout prep. Returns per-core input maps."""
    bf16, f8e4, f8e5 = _npdt()
    x2d = np.ascontiguousarray(np.asarray(x, np.float32).reshape(T, C))
    Wq = np.asarray(Wq, np.float32)
    Wk = np.asarray(Wk, np.float32)
    Wv = np.asarray(Wv, np.float32)
    Wo = np.asarray(Wo, np.float32)
    q_norm_w = np.asarray(q_norm_w, np.float32)
    k_norm_w = np.asarray(k_norm_w, np.float32)
    pos = np.asarray(input_pos, np.float32)

    # x^T in bf16 k-tile layout: [c_loc, (kt, t)]
    xt_host = np.ascontiguousarray(
        x2d.T.reshape(NK, 128, T).transpose(1, 0, 2).reshape(128, -1)
    ).astype(bf16)

    # interleaved head-dim permutation: [0, 64, 1, 65, ...]
    perm = np.empty(128, np.int64)
    perm[0::2] = np.arange(64)
    perm[1::2] = np.arange(64) + 64

    # rope tables in interleaved layout (sign of the rotate-half folded in)
    inv_freq = (THETA ** (-(np.arange(0, D, 2, dtype=np.float32)) / D)).astype(
        np.float32
    )
    fr = pos[:, None] * inv_freq[None, :]  # [T, 64]
    cos = np.cos(fr).astype(np.float32).T  # [64, T]
    sin = np.sin(fr).astype(np.float32).T
    cos_il = np.empty((128, T), np.float32)
    cos_il[0::2] = cos
    cos_il[1::2] = cos
    sin_eff = np.empty((128, T), np.float32)
    sin_eff[0::2] = -sin
    sin_eff[1::2] = sin
    cos_il = np.ascontiguousarray(cos_il).astype(bf16)
    sin_eff = np.ascontiguousarray(sin_eff).astype(bf16)

    # identity pair (slab1 = 0) for the DoubleRow mask-bias matmul
    ident8_host = np.zeros((128, 2, 128), np.float32)
    ident8_host[:, 0, :] = np.eye(128)
    ident8_host = ident8_host.reshape(128, -1).astype(f8e4)

    # causal mask bias tiles: pattern P covers diag blocks u = 2P, 2P+1;
    # bias_u[p, t] = 0 if t - p >= 128*u else MASKVAL (slab 1 stays zero)
    tt, pp = np.meshgrid(np.arange(512), np.arange(128), indexing="xy")
    mask_host = np.zeros((128, 2, 2, 2, 512), np.float32)
    for P in range(2):
        for w in range(2):
            u = 2 * P + w
            keep = (tt - pp - 128 * u) >= 0
            mask_host[:, P, w, 0, :] = np.where(keep, 0.0, MASKVAL)
    mask_host = mask_host.reshape(128, -1).astype(f8e4)

    Wq4 = Wq.reshape(N_HEAD, D, C) * q_norm_w[None, :, None]
    Wk4 = Wk.reshape(N_KV, D, C) * k_norm_w[None, :, None]
    Wv4 = Wv.reshape(N_KV, D, C)

    in_maps = []
    for c in range(NCORES):
        g = c // 2
        # [tgt, d, kt, c_loc] -> [c_loc, tgt, kt, d]; tgt 0..3 = q, 4 = k
        Wqk = np.concatenate(
            [Wq4[HPC * c:HPC * (c + 1)][:, perm, :], Wk4[g][None, perm, :]], axis=0
        )  # [5, 128, C]
        wq_host = np.ascontiguousarray(
            Wqk.reshape(5, 128, NK, 128).transpose(3, 0, 2, 1).reshape(128, -1)
        ).astype(bf16)
        wv_host = np.ascontiguousarray(
            Wv4[g].reshape(128, NK, 128).transpose(2, 1, 0).reshape(128, -1)
        ).astype(bf16)
        # woT[d, (h, rt, rr)] = Wo[128 rt + rr, 128*(4c + h) + d]
        wo_host = np.ascontiguousarray(
            Wo[:, 512 * c:512 * (c + 1)]
            .reshape(16, 128, HPC, 128).transpose(3, 2, 0, 1).reshape(128, -1)
        ).astype(bf16)
        in_maps.append(
            {
                "xtb": xt_host,
                "wqb": wq_host,
                "wvb": wv_host,
                "wob": wo_host,
                "cost": cos_il,
                "sint": sin_eff,
                "ident8p": ident8_host,
                "mask8p": mask_host,
            }
        )
    return in_maps


def kernel(x, input_pos, Wq, Wk, Wv, Wo, q_norm_w, k_norm_w):
    run = _get_runner()
    in_maps = make_inputs(x, input_pos, Wq, Wk, Wv, Wo, q_norm_w, k_norm_w)
    results = run(in_maps)
    out = np.empty((1, T, C), np.float32)
    for c in range(NCORES):
        out[0][:, 256 * c:256 * (c + 1)] = results[c]["outT"].T
    return out


# revision 66
# speedup vs baseline: 1.0228x; 1.0228x over previous
"""Trainium2 Bass kernel for causal self-attention (GQA, RoPE, q/k-RMSNorm).

Sharding: tensor-parallel over heads across 8 cores.
  - core c owns q-heads [4c, 4c+4) and kv-head c//2
  - x^T is pre-transposed on the host and DMA'd in bf16, so there is no
    on-device transpose phase; V is projected directly into its natural
    [S, D] layout by using x^T as the matmul lhsT
  - all projections/attention matmuls run in bf16 (fp32 PSUM); the causal
    mask is injected as an exact fp8 DoubleRow PE bias matmul (0 / -240
    tiles) accumulated into the score PSUM before the exp
  - QKV streams through a 2-bank PSUM scratch so the attention pools can
    stay resident; rmsnorm+rope is restructured so the reduce chain
    (sq->ssq->sqrt->recip->PE broadcast) and the rope chain (on raw
    values) run independently and meet in one final multiply
  - o_proj is a row-sharded partial (woT blocks per local head); each
    t-chunk's [2048, 512] partial is summed across cores by its own bf16
    ReduceScatter, whose rank shard is exactly this core's outT columns,
    so chunk j's collective overlaps chunk j+1's compute
"""

import sys

sys.path.insert(0, "/opt/trn_rl_repo")

from contextlib import ExitStack

import numpy as np

import bass_rust
import concourse.bass as bass
import concourse.mybir as mybir
from concourse import tile

F32 = mybir.dt.float32
F32R = mybir.dt.float32r
BF16 = mybir.dt.bfloat16
FP8 = mybir.dt.float8e4
FP8E5 = mybir.dt.float8e5
DR = mybir.MatmulPerfMode.DoubleRow

N_HEAD = 32
N_KV = 4
D = 128
C = 2048
T = 2048
NCORES = 8
HPC = N_HEAD // NCORES  # q heads per core = 4
THETA = 1000000.0
EPS = 1e-6
SCALE = 1.0 / np.sqrt(128.0)
MASKVAL = -240.0  # pre-scale bias; * SCALE = -21 -> exp ~ 6e-10 -> fp8 0

NT = T // 512  # 4 T-chunks of 512
NK = 16  # contraction tiles of 128
NS = T // 128  # 16 s-blocks of 128

# stream_shuffle swaps within each 32-partition quadrant; adjacent-pair swap
SWAP_MASK = [i ^ 1 for i in range(32)]

_NPDT = None


def _npdt():
    global _NPDT
    if _NPDT is None:
        import ml_dtypes

        _NPDT = (
            np.dtype(ml_dtypes.bfloat16),
            np.dtype(ml_dtypes.float8_e4m3),
            np.dtype(ml_dtypes.float8_e5m2),
        )
    return _NPDT


def split_multiwaits(nc):
    """The walrus build in this container supports one sync-wait per
    instruction; hoist extra waits onto NOPs inserted before the offender."""
    ctr = 0
    for f in nc.m.functions:
        for bb in f.blocks:
            new_insts = []
            changed = False
            for inst in bb.instructions:
                si = inst.sync_info
                if si is not None and si.on_wait and len(si.on_wait) > 1:
                    waits = list(si.on_wait)
                    for w in waits[:-1]:
                        ctr += 1
                        nop = bass_rust.InstNoOp(name=f"splitw-{ctr}", ins=[], outs=[])
                        nop.engine = inst.engine
                        nop.sync_info = bass_rust.SyncInfo(on_wait=[w], on_update=[])
                        new_insts.append(nop)
                    inst.sync_info = bass_rust.SyncInfo(
                        on_wait=[waits[-1]], on_update=list(si.on_update or [])
                    )
                    changed = True
                new_insts.append(inst)
            if changed:
                bb.instructions = new_insts


def _bcast(ap, n):
    """Insert a stride-0 free dim of extent n after the partition dim."""
    import dataclasses
    assert len(ap.ap) == 2
    return dataclasses.replace(ap, ap=[ap.ap[0], [0, n], ap.ap[1]])


def build_program(bench_reps=0, phases="ABDF", split=True):
    nc = bass.Bass("TRN2", target_bir_lowering=False, debug=False, num_devices=NCORES)

    xtb = nc.declare_dram_parameter("xtb", [128, NK * T], BF16, isOutput=False)
    wqb = nc.declare_dram_parameter(
        "wqb", [128, 5 * NK * 128], BF16, isOutput=False
    )
    wvb = nc.declare_dram_parameter("wvb", [128, NK * 128], BF16, isOutput=False)
    wob = nc.declare_dram_parameter(
        "wob", [128, HPC * 16 * 128], BF16, isOutput=False
    )
    cost = nc.declare_dram_parameter("cost", [128, T], BF16, isOutput=False)
    sint = nc.declare_dram_parameter("sint", [128, T], BF16, isOutput=False)
    ident8p = nc.declare_dram_parameter("ident8p", [128, 2 * 128], FP8, isOutput=False)
    mask8p = nc.declare_dram_parameter(
        "mask8p", [128, 2 * 2 * 2 * 512], FP8, isOutput=False
    )
    outT = nc.declare_dram_parameter("outT", [256, T], BF16, isOutput=True)

    rg = [list(range(NCORES))]
    collectives = bench_reps == 0

    with tile.TileContext(nc) as tc, ExitStack() as ctx:
        const = ctx.enter_context(tc.tile_pool(name="const", bufs=1))
        wpool = ctx.enter_context(tc.tile_pool(name="wpool", bufs=1))
        act = ctx.enter_context(tc.tile_pool(name="act", bufs=1))
        dram = ctx.enter_context(tc.tile_pool(name="dram", bufs=1, space="DRAM"))

        # ---- constants ----
        ones_col = const.tile([128, 1], BF16)
        nc.vector.memset(ones_col[:], 1.0)
        ones_row = const.tile([1, 128], BF16)
        nc.vector.memset(ones_row[:], 1.0)
        eps_col = const.tile([128, 1], F32)
        nc.vector.memset(eps_col[:], EPS)
        ident8 = const.tile([128, 2, 128], FP8)
        nc.sync.dma_start(ident8[:], ident8p[:, :])
        mask8 = const.tile([128, 2, 2, 2, 512], FP8)
        nc.sync.dma_start(mask8[:], mask8p[:, :])

        # ---- resident weights / tables ----
        wq_sb = wpool.tile([128, 5, NK, 128], BF16)  # q0..q3 + k
        for o in range(5):
            nc.sync.dma_start(
                wq_sb[:, o, :, :], wqb[:, o * NK * 128:(o + 1) * NK * 128]
            )
        xt_sb = wpool.tile([128, NK, T], BF16)
        for kt in range(NK):
            nc.sync.dma_start(
                xt_sb[:, kt, :], xtb[:, kt * T:(kt + 1) * T]
            )
        wv_sb = wpool.tile([128, NK, 128], BF16)
        nc.sync.dma_start(wv_sb[:], wvb[:, :])
        wo_sb = wpool.tile([128, HPC, 16, 128], BF16)
        nc.sync.dma_start(wo_sb[:], wob[:, :])
        cos_sb = wpool.tile([128, T], BF16)
        nc.sync.dma_start(cos_sb[:], cost[:, :])
        sin_sb = wpool.tile([128, T], BF16)
        nc.sync.dma_start(sin_sb[:], sint[:, :])

        # ---- persistent activations ----
        kT = act.tile([128, T], BF16)
        vN = act.tile([128, NS, 128], BF16)  # natural [S,D] s-blocks
        yT = act.tile([128, HPC, T], BF16)

        # per-t-chunk partial-o + ReduceScatter buffers: a full-r RS of
        # [2048, 512] hands core c rows [256c, 256c+256) = its outT slice
        o_part = [
            dram.tile([16 * 128, 512], BF16, name=f"opart{j}") for j in range(NT)
        ]
        rs_out = [
            dram.tile([256, 512], BF16, name=f"rsout{j}") for j in range(NT)
        ]

        def qkv_chunk(j, pb_sb, ps_scr):
            js = slice(j * 512, (j + 1) * 512)
            raw = pb_sb.tile([128, 5, 512], BF16, tag="raw", bufs=2)
            for o in range(5):  # q0..q3, k -> [d, t] via 2-bank streaming
                acc = ps_scr.tile([128, 512], F32, tag="s", name=f"acc{o}")
                for kt in range(NK):
                    nc.tensor.matmul(
                        acc[:], wq_sb[:, o, kt, :], xt_sb[:, kt, js],
                        start=(kt == 0), stop=(kt == NK - 1),
                    )
                nc.scalar.activation(
                    raw[:, o, :], acc[:], mybir.ActivationFunctionType.Copy
                )
            # v directly in natural [s, d] layout: lhsT = x^T, rhs = Wv^T
            vacc = ps_scr.tile([128, 4, 128], F32, tag="v", bufs=1)
            for u in range(4):
                for kt in range(NK):
                    nc.tensor.matmul(
                        vacc[:, u, :],
                        xt_sb[:, kt, j * 512 + u * 128:j * 512 + (u + 1) * 128],
                        wv_sb[:, kt, :],
                        start=(kt == 0), stop=(kt == NK - 1),
                    )
            nc.vector.tensor_copy(vN[:, 4 * j:4 * j + 4, :], vacc[:])
            return raw

        def norms_reduce(j, raw, pc_sb, ps_scr):
            # sq -> ssq -> sqrt -> recip; emitted before the deferred o_proj
            # so the cross-engine latency hides under its PE work
            n = 5
            sqf = pc_sb.tile([128, n, 512], BF16, tag="sq", bufs=1)
            for i in range(n):
                nc.vector.tensor_mul(sqf[:, i, :], raw[:, i, :], raw[:, i, :])
            ssq = [ps_scr.tile([128, 512], F32, tag="s", name=f"ssq{i}")
                   for i in range(n)]
            for i in range(n):
                nc.tensor.matmul(ssq[i][0:1, :], ones_col[:], sqf[:, i, :])
            rms = pc_sb.tile([1, n, 512], BF16, tag="rms", bufs=2)
            for i in range(n):
                nc.scalar.activation(
                    rms[:, i, :], ssq[i][0:1, :],
                    mybir.ActivationFunctionType.Sqrt,
                    scale=1.0 / 128.0, bias=eps_col[0:1, :],
                )
            rinv = pc_sb.tile([1, n, 512], BF16, tag="ri", bufs=1)
            with nc.allow_low_precision(reason="feeds PE broadcast"):
                nc.vector.reciprocal(rinv[:], rms[:])
            return rinv

        def norms_finish(j, raw, rinv, pc_sb, ps_scr):
            # rope on raw values + one final multiply from the rb broadcast
            js = slice(j * 512, (j + 1) * 512)
            n = 5
            rb = [ps_scr.tile([128, 512], F32, tag="s", name=f"rb{i}")
                  for i in range(n)]
            for i in range(n):
                nc.tensor.matmul(rb[i][:], ones_row[:], rinv[:, i, :])
            qs = pc_sb.tile([128, n, 512], BF16, tag="qs", bufs=1)
            for i in range(n):
                nc.vector.stream_shuffle(qs[:, i, :], raw[:, i, :], mask=SWAP_MASK)
            t1 = pc_sb.tile([128, n, 512], BF16, tag="t1", bufs=1)
            for i in range(n):
                nc.gpsimd.tensor_mul(t1[:, i, :], raw[:, i, :], cos_sb[:, js])
            t2 = pc_sb.tile([128, n, 512], BF16, tag="t2", bufs=1)
            for i in range(n):
                nc.vector.tensor_mul(t2[:, i, :], qs[:, i, :], sin_sb[:, js])
            rr = pc_sb.tile([128, n, 512], BF16, tag="rr", bufs=1)
            nc.vector.tensor_add(rr[:], t1[:], t2[:])
            qTc = pc_sb.tile([128, HPC, 512], BF16, tag="qtc", bufs=2)
            for i in range(HPC):
                nc.vector.tensor_mul(qTc[:, i, :], rr[:, i, :], rb[i][:])
            nc.vector.tensor_mul(kT[:, js], rr[:, 4, :], rb[4][:])
            return qTc

        def attn_chunk(j, qTc, pd_sb, ps_att, ps_scr):
            js = slice(j * 512, (j + 1) * 512)
            npair = 2 * j + 2
            for h in range(HPC):
                ps_y = ps_att.tile([128, 512], F32, tag="psy")
                ps_den = ps_scr.tile([128, 512], F32, tag="s", name=f"den{h}")
                for p in range(npair):
                    ps_s = ps_att.tile([128, 2, 512], F32, tag="pss", bufs=2)
                    diag = p >= 2 * j
                    for w in range(2):
                        i = 2 * p + w
                        if diag:
                            nc.tensor.matmul(
                                ps_s[:, w, :], ident8[:],
                                mask8[:, p - 2 * j, w, :, :],
                                start=True, stop=False, perf_mode=DR,
                            )
                        nc.tensor.matmul(
                            ps_s[:, w, :],
                            kT[:, i * 128:(i + 1) * 128], qTc[:, h, :],
                            start=not diag, stop=True,
                        )
                    etb = pd_sb.tile([128, 2, 512], BF16, tag="etb", bufs=2)
                    nc.scalar.activation(
                        etb[:], ps_s[:], mybir.ActivationFunctionType.Exp,
                        scale=float(SCALE),
                    )
                    for w in range(2):
                        st = dict(
                            start=(p == 0 and w == 0),
                            stop=(p == npair - 1 and w == 1),
                        )
                        nc.tensor.matmul(
                            ps_y[:], vN[:, 2 * p + w, :], etb[:, w, :], **st
                        )
                        nc.tensor.matmul(
                            ps_den[0:1, :], ones_col[:], etb[:, w, :], **st
                        )
                rd = pd_sb.tile([1, 512], BF16, tag="rd", bufs=1)
                with nc.allow_low_precision(reason="feeds PE broadcast"):
                    nc.vector.reciprocal(rd[:], ps_den[0:1, :])
                ps_rb = ps_scr.tile([128, 512], F32, tag="s", name=f"psrb{h}")
                nc.tensor.matmul(ps_rb[:], ones_row[:], rd[:])
                rbc = pd_sb.tile([128, 512], BF16, tag="rbcd", bufs=2)
                nc.vector.tensor_copy(rbc[:], ps_rb[:])
                nc.vector.tensor_mul(yT[:, h, js], ps_y[:], rbc[:])

        def oproj_chunk(j, pf_sb, ps_scr, rts=range(16), rs=True):
            js = slice(j * 512, (j + 1) * 512)
            for rt in rts:
                ps = ps_scr.tile([128, 512], F32, tag="s", name=f"pso{rt}")
                for h in range(HPC):
                    nc.tensor.matmul(
                        ps[:], wo_sb[:, h, rt, :], yT[:, h, js],
                        start=(h == 0), stop=(h == HPC - 1),
                    )
                ob = pf_sb.tile([128, 512], BF16, tag="ob", bufs=2)
                if rt % 2 == 0:
                    nc.vector.tensor_copy(ob[:], ps[:])
                else:
                    nc.scalar.activation(
                        ob[:], ps[:], mybir.ActivationFunctionType.Copy
                    )
                nc.sync.dma_start(o_part[j][rt * 128:(rt + 1) * 128, :], ob[:])
            if rs and collectives:
                nc.gpsimd.collective_compute(
                    "ReduceScatter", mybir.AluOpType.add, replica_groups=rg,
                    ins=[o_part[j][:].opt()], outs=[rs_out[j][:].opt()],
                )

        def body():
            with tc.tile_pool(name="pb_sb", bufs=2) as pb_sb, \
                 tc.tile_pool(name="pc_sb", bufs=2) as pc_sb, \
                 tc.tile_pool(name="pd_sb", bufs=3) as pd_sb, \
                 tc.tile_pool(name="ps_att", bufs=1, space="PSUM") as ps_att, \
                 tc.tile_pool(name="ps_scr", bufs=2, space="PSUM") as ps_scr:
                for j in range(NT):
                    raw = qkv_chunk(j, pb_sb, ps_scr)
                    rinv = norms_reduce(j, raw, pc_sb, ps_scr)
                    if "D" in phases and "F" in phases and j > 0:
                        oproj_chunk(j - 1, pd_sb, ps_scr, range(0, 8), rs=False)
                    qTc = norms_finish(j, raw, rinv, pc_sb, ps_scr)
                    if "D" in phases and "F" in phases and j > 0:
                        oproj_chunk(j - 1, pd_sb, ps_scr, range(8, 16))
                    if "D" not in phases:
                        continue
                    attn_chunk(j, qTc, pd_sb, ps_att, ps_scr)
                if "D" in phases and "F" in phases:
                    oproj_chunk(NT - 1, pd_sb, ps_scr)

            # bounce the RS shards into outT
            if "F" not in phases or "D" not in phases:
                return
            with tc.tile_pool(name="po_sb", bufs=2) as po_sb:
                for j in range(NT):
                    js = slice(j * 512, (j + 1) * 512)
                    rsb = po_sb.tile([128, 2, 512], BF16, tag="rsb")
                    for m in range(2):
                        nc.sync.dma_start(
                            rsb[:, m, :], rs_out[j][m * 128:(m + 1) * 128, :]
                        )
                    for m in range(2):
                        nc.sync.dma_start(
                            outT[m * 128:(m + 1) * 128, js], rsb[:, m, :]
                        )

        if bench_reps:
            with tc.For_i(0, bench_reps, 1):
                body()
        else:
            body()

    if split:
        split_multiwaits(nc)
    return nc


# ---------------------------------------------------------------------------
# host side
# ---------------------------------------------------------------------------

_RUNNER_CACHE = None


def _make_runner(nc, n_cores=NCORES):
    """Build the sharded jit once; returns run(in_maps) -> list of out dicts."""
    import jax
    from jax.sharding import Mesh, NamedSharding, PartitionSpec
    from jax.experimental.shard_map import shard_map
    from concourse import bass2jax
    from concourse.bass2jax import _bass_exec_p, partition_id_tensor

    bass2jax.install_neuronx_cc_hook()

    partition_name = nc.partition_id_tensor.name if nc.partition_id_tensor else None
    in_names, out_names, out_avals, zero_outs = [], [], [], []
    for alloc in nc.m.functions[0].allocations:
        if not isinstance(alloc, mybir.MemoryLocationSet):
            continue
        name = alloc.memorylocations[0].name
        if alloc.kind == "ExternalInput":
            if name != partition_name:
                in_names.append(name)
        elif alloc.kind == "ExternalOutput":
            out_names.append(name)
            shape = tuple(alloc.tensor_shape)
            dtype = mybir.dt.np(alloc.dtype)
            out_avals.append(jax.core.ShapedArray(shape, dtype))
            zero_outs.append(np.zeros(shape, dtype))
    n_params = len(in_names)
    n_outs = len(out_avals)
    all_in_names = list(in_names) + list(out_names)
    if partition_name is not None:
        all_in_names.append(partition_name)
    donate = tuple(range(n_params, n_params + n_outs))

    def _body(*args):
        operands = list(args)
        if partition_name is not None:
            operands.append(partition_id_tensor())
        outs = _bass_exec_p.bind(
            *operands,
            out_avals=tuple(out_avals),
            in_names=tuple(all_in_names),
            out_names=tuple(out_names),
            lowering_input_output_aliases=(),
            sim_require_finite=True,
            sim_require_nnan=True,
            nc=nc,
        )
        return tuple(outs)

    devices = jax.devices()[:n_cores]
    mesh = Mesh(np.asarray(devices), ("core",))
    sharded = jax.jit(
        shard_map(
            _body, mesh=mesh,
            in_specs=(PartitionSpec("core"),) * (n_params + n_outs),
            out_specs=(PartitionSpec("core"),) * n_outs,
            check_rep=False,
        ),
        donate_argnums=donate,
        keep_unused=True,
    )
    shard = NamedSharding(mesh, PartitionSpec("core"))
    zshapes = [((n_cores * z.shape[0],) + z.shape[1:], z.dtype) for z in zero_outs]

    def run(in_maps):
        concat_in = [
            jax.device_put(
                np.concatenate(
                    [np.asarray(in_maps[c][n]) for c in range(n_cores)], axis=0
                ),
                shard,
            )
            for n in in_names
        ]
        zs = [jax.device_put(np.zeros(s, d), shard) for s, d in zshapes]
        outs = sharded(*concat_in, *zs)
        return [
            {
                name: np.asarray(outs[i]).reshape(n_cores, *out_avals[i].shape)[c]
                for i, name in enumerate(out_names)
            }
            for c in range(n_cores)
        ]

    return run


def _get_runner():
    global _RUNNER_CACHE
    if _RUNNER_CACHE is None:
        _RUNNER_CACHE = _make_runner(build_program())
    return _RUNNER_CACHE


def make_inputs(x, input_pos, Wq, Wk, Wv, Wo, q_norm_w, k_norm_w):
    """Host-side sharding / layout prep. Returns per-core input maps."""
    bf16, f8e4, f8e5 = _npdt()
    x2d = np.ascontiguousarray(np.asarray(x, np.float32).reshape(T, C))
    Wq = np.asarray(Wq, np.float32)
    Wk = np.asarray(Wk, np.float32)
    Wv = np.asarray(Wv, np.float32)
    Wo = np.asarray(Wo, np.float32)
    q_norm_w = np.asarray(q_norm_w, np.float32)
    k_norm_w = np.asarray(k_norm_w, np.float32)
    pos = np.asarray(input_pos, np.float32)

    # x^T in bf16 k-tile layout: [c_loc, (kt, t)]
    xt_host = np.ascontiguousarray(
        x2d.T.reshape(NK, 128, T).transpose(1, 0, 2).reshape(128, -1)
    ).astype(bf16)

    # interleaved head-dim permutation: [0, 64, 1, 65, ...]
    perm = np.empty(128, np.int64)
    perm[0::2] = np.arange(64)
    perm[1::2] = np.arange(64) + 64

    # rope tables in interleaved layout (sign of the rotate-half folded in)
    inv_freq = (THETA ** (-(np.arange(0, D, 2, dtype=np.float32)) / D)).astype(
        np.float32
    )
    fr = pos[:, None] * inv_freq[None, :]  # [T, 64]
    cos = np.cos(fr).astype(np.float32).T  # [64, T]
    sin = np.sin(fr).astype(np.float32).T
    cos_il = np.empty((128, T), np.float32)
    cos_il[0::2] = cos
    cos_il[1::2] = cos
    sin_eff = np.empty((128, T), np.float32)
    sin_eff[0::2] = -sin
    sin_eff[1::2] = sin
    cos_il = np.ascontiguousarray(cos_il).astype(bf16)
    sin_eff = np.ascontiguousarray(sin_eff).astype(bf16)

    # identity pair (slab1 = 0) for the DoubleRow mask-bias matmul
    ident8_host = np.zeros((128, 2, 128), np.float32)
    ident8_host[:, 0, :] = np.eye(128)
    ident8_host = ident8_host.reshape(128, -1).astype(f8e4)

    # causal mask bias tiles: pattern P covers diag blocks u = 2P, 2P+1;
    # bias_u[p, t] = 0 if t - p >= 128*u else MASKVAL (slab 1 stays zero)
    tt, pp = np.meshgrid(np.arange(512), np.arange(128), indexing="xy")
    mask_host = np.zeros((128, 2, 2, 2, 512), np.float32)
    for P in range(2):
        for w in range(2):
            u = 2 * P + w
            keep = (tt - pp - 128 * u) >= 0
            mask_host[:, P, w, 0, :] = np.where(keep, 0.0, MASKVAL)
    mask_host = mask_host.reshape(128, -1).astype(f8e4)

    Wq4 = Wq.reshape(N_HEAD, D, C) * q_norm_w[None, :, None]
    Wk4 = Wk.reshape(N_KV, D, C) * k_norm_w[None, :, None]
    Wv4 = Wv.reshape(N_KV, D, C)

    in_maps = []
    for c in range(NCORES):
        g = c // 2
        # [tgt, d, kt, c_loc] -> [c_loc, tgt, kt, d]; tgt 0..3 = q, 4 = k
        Wqk = np.concatenate(
            [Wq4[HPC * c:HPC * (c + 1)][:, perm, :], Wk4[g][None, perm, :]], axis=0
        )  # [5, 128, C]
        wq_host = np.ascontiguousarray(
            Wqk.reshape(5, 128, NK, 128).transpose(3, 0, 2, 1).reshape(128, -1)
        ).astype(bf16)
        wv_host = np.ascontiguousarray(
            Wv4[g].reshape(128, NK, 128).transpose(2, 1, 0).reshape(128, -1)
        ).astype(bf16)
        # woT[d, (h, rt, rr)] = Wo[128 rt + rr, 128*(4c + h) + d]
        wo_host = np.ascontiguousarray(
            Wo[:, 512 * c:512 * (c + 1)]
            .reshape(16, 128, HPC, 128).transpose(3, 2, 0, 1).reshape(128, -1)
        ).astype(bf16)
        in_maps.append(
            {
                "xtb": xt_host,
                "wqb": wq_host,
                "wvb": wv_host,
                "wob": wo_host,
                "cost": cos_il,
                "sint": sin_eff,
                "ident8p": ident8_host,
                "mask8p": mask_host,
            }
        )
    return in_maps


def kernel(x, input_pos, Wq, Wk, Wv, Wo, q_norm_w, k_norm_w):
    run = _get_runner()
    in_maps = make_inputs(x, input_pos, Wq, Wk, Wv, Wo, q_norm_w, k_norm_w)
    results = run(in_maps)
    out = np.empty((1, T, C), np.float32)
    for c in range(NCORES):
        out[0][:, 256 * c:256 * (c + 1)] = results[c]["outT"].T
    return out
